# revision 7
# baseline (speedup 1.0000x reference)
"""Trainium2 Bass kernel for nn_BatchSoftmaxNomax (batch contrastive softmax loss).

Math: scores[b,c,n,f] = <ner[b,n,:], face[c,f,:]>, logits = scores.mean((n,f)),
loss = -mean_b log_softmax(logits)[b,b].
Since the span-means are linear, logits[b,c] = <mean_n ner[b], mean_f face[c]>,
so the O(B^2*N^2*D) einsum collapses to two mean-reductions + a [B,D]x[D,B] matmul.

Sharding (8 cores, batch-sharded): each core takes 32 rows of ner and face,
computes span-means on-device via PE matmuls against a 1/32 selection matrix,
AllGathers the face means (32x512 -> 256x512), transposes means with the PE,
computes its 32x256 logit rows, then exp+rowsum (ACT fused accumulate) and the
diagonal logit (DVE masked reduce). Host combines: loss = -mean(diag - log(rowsum)).
"""

import numpy as np
from contextlib import ExitStack

B = 256      # global batch
N1 = 32      # ner spans
N2 = 32      # face spans
D = 512      # embed dim
M = 8        # cores
BL = B // M  # local batch rows per core (32)
R = BL * N1  # rows of the flattened local slice (1024)
PJ = R // 128  # rows per partition in the [128, PJ, D] DMA view (8)
NCH = 4      # DMA chunks per input tensor
JPC = PJ // NCH

_CACHE = {}


def _emit(ctx, tc, out, ner, face, sel, ident, dmask, ag_shared=True):
    import concourse.bass as bass
    from concourse import mybir

    nc = tc.nc
    f32 = mybir.dt.float32
    f32r = mybir.dt.float32r
    AF = mybir.ActivationFunctionType
    ALU = mybir.AluOpType

    consts = ctx.enter_context(tc.tile_pool(name="consts", bufs=1))
    chunks = ctx.enter_context(tc.tile_pool(name="chunks", bufs=3))
    sbuf = ctx.enter_context(tc.tile_pool(name="work", bufs=1))
    mpsum = ctx.enter_context(tc.tile_pool(name="mpsum", bufs=2, space="PSUM"))
    tpsum = ctx.enter_context(tc.tile_pool(name="tpsum", bufs=2, space="PSUM"))
    lpsum = ctx.enter_context(tc.tile_pool(name="lpsum", bufs=1, space="PSUM"))
    dram = ctx.enter_context(tc.tile_pool(name="dram", bufs=1, space="DRAM"))

    sel_sb = consts.tile([128, BL], f32r)
    nc.gpsimd.dma_start(sel_sb[:], sel)
    ident_sb = consts.tile([128, 128], f32)
    nc.sync.dma_start(ident_sb[:], ident)
    dmask_sb = consts.tile([BL, B], f32)
    nc.sync.dma_start(dmask_sb[:], dmask)

    # Warm the ACT exp table set while DMAs stream.
    warm_in = sbuf.tile([1, 1], f32)
    nc.vector.memset(warm_in[:], 0.0)
    warm_out = sbuf.tile([1, 1], f32)
    nc.scalar.activation(warm_out[:], warm_in[:], AF.Exp)

    def mean_of(src_ap, tag):
        # psum[m, d] = sum_{p,j} sel[p, m] * src[8p + j, d] = (1/32) sum_n src[32m + n, d]
        view = src_ap.rearrange("(p j) d -> p j d", p=128)
        ps = mpsum.tile([BL, D], f32, tag="mean", bufs=2)
        for q in range(NCH):
            t = chunks.tile([128, JPC, D], f32r, tag="chunk", bufs=3)
            nc.gpsimd.dma_start(t[:], view[:, q * JPC:(q + 1) * JPC, :])
            for jj in range(JPC):
                j = q * JPC + jj
                nc.tensor.matmul(
                    ps[:],
                    sel_sb[:],
                    t[:, jj, :],
                    start=(j == 0),
                    stop=(j == PJ - 1),
                )
        mn = sbuf.tile([BL, D], f32, tag="mean_sb_" + tag)
        nc.vector.tensor_copy(mn[:], ps[:])
        return mn

    fm_sb = mean_of(face, "fm")

    fm_dram = dram.tile([BL, D], f32)
    nc.sync.dma_start(fm_dram[:], fm_sb[:])
    g_dram = dram.tile([B, D], f32, addr_space="Shared" if ag_shared else "Local")
    nc.gpsimd.collective_compute(
        "AllGather",
        ALU.bypass,
        replica_groups=[list(range(M))],
        ins=[fm_dram[:].opt()],
        outs=[g_dram[:].opt()],
    )

    nm_sb = mean_of(ner, "nm")

    # nmT[128, 4*32]: chunk k holds transpose of nm[:, 128k:128k+128]
    nmT_ps = tpsum.tile([128, 128], f32, tag="nmT", bufs=1)
    for k in range(4):
        nc.tensor.transpose(
            nmT_ps[:, k * BL:(k + 1) * BL],
            nm_sb[:, k * 128:(k + 1) * 128],
            ident_sb[:BL, :BL],
        )
    nmT_sb = sbuf.tile([128, 128], f32r)
    nc.vector.tensor_copy(nmT_sb[:], nmT_ps[:])

    # Load gathered face means and transpose to fmT_k [128 d', 256 c]
    g_sb = []
    for h in range(2):
        gt = chunks.tile([128, D], f32, tag="gtile", bufs=2)
        nc.sync.dma_start(gt[:], g_dram[:][h * 128:(h + 1) * 128, :])
        g_sb.append(gt)
    fmT_sb = []
    for k in range(4):
        fps = tpsum.tile([128, B], f32, tag="fmT_ps", bufs=2)
        for h in range(2):
            nc.tensor.transpose(
                fps[:, h * 128:(h + 1) * 128],
                g_sb[h][:, k * 128:(k + 1) * 128],
                ident_sb[:],
            )
        ft = sbuf.tile([128, B], f32r, tag=f"fmT{k}")
        nc.vector.tensor_copy(ft[:], fps[:])
        fmT_sb.append(ft)

    # logits[b, c] = sum_k nmT_k^T @ fmT_k
    lg = lpsum.tile([BL, B], f32)
    for k in range(4):
        nc.tensor.matmul(
            lg[:],
            nmT_sb[:, k * BL:(k + 1) * BL],
            fmT_sb[k][:],
            start=(k == 0),
            stop=(k == 3),
        )

    # res[:, 1] = sum_c exp(logits); res[:, 0] = logits[b, 32*core + b] via mask.
    # NB: InstTensorTensorReduce and 2-input DVE reads from PSUM wedge the device
    # here, so: ACT exp straight off PSUM (fine), and diag via copy -> mult -> reduce.
    res = sbuf.tile([BL, 2], f32)
    e_sb = sbuf.tile([BL, B], f32)
    nc.scalar.activation(e_sb[:], lg[:], AF.Exp, accum_out=res[:, 1:2])
    lg_sb = sbuf.tile([BL, B], f32)
    nc.vector.tensor_copy(lg_sb[:], lg[:])
    dscratch = sbuf.tile([BL, B], f32)
    nc.vector.tensor_tensor(out=dscratch[:], in0=lg_sb[:], in1=dmask_sb[:], op=ALU.mult)
    nc.vector.tensor_reduce(
        out=res[:, 0:1], in_=dscratch[:], axis=mybir.AxisListType.X, op=ALU.add
    )
    nc.sync.dma_start(out, res[:])


def _build_nc():
    import concourse.tile as tile
    from concourse import bacc, mybir

    f32 = mybir.dt.float32
    nc = bacc.Bacc("TRN2", target_bir_lowering=False, debug=False, num_devices=M)

    ner = nc.dram_tensor("ner", [R, D], f32, kind="ExternalInput").ap()
    face = nc.dram_tensor("face", [R, D], f32, kind="ExternalInput").ap()
    sel = nc.dram_tensor("sel", [128, BL], f32, kind="ExternalInput").ap()
    ident = nc.dram_tensor("ident", [128, 128], f32, kind="ExternalInput").ap()
    dmask = nc.dram_tensor("dmask", [BL, B], f32, kind="ExternalInput").ap()
    out = nc.dram_tensor("out", [BL, 2], f32, kind="ExternalOutput").ap()

    with tile.TileContext(nc) as tc:
        with ExitStack() as ctx:
            _emit(ctx, tc, out, ner, face, sel, ident, dmask)
    nc.compile()
    return nc


def get_nc():
    if "nc" not in _CACHE:
        _CACHE["nc"] = _build_nc()
    return _CACHE["nc"]


def build_in_maps(face_j, ner_j):
    face_j = np.ascontiguousarray(face_j, dtype=np.float32)
    ner_j = np.ascontiguousarray(ner_j, dtype=np.float32)
    sel = np.zeros((128, BL), np.float32)
    sel[np.arange(128), np.arange(128) // 4] = 1.0 / N1
    ident = np.eye(128, dtype=np.float32)
    in_maps = []
    for c in range(M):
        dm = np.zeros((BL, B), np.float32)
        dm[np.arange(BL), c * BL + np.arange(BL)] = 1.0
        in_maps.append(
            {
                "ner": np.ascontiguousarray(
                    ner_j[c * BL:(c + 1) * BL].reshape(R, D)
                ),
                "face": np.ascontiguousarray(
                    face_j[c * BL:(c + 1) * BL].reshape(R, D)
                ),
                "sel": sel,
                "ident": ident,
                "dmask": dm,
            }
        )
    return in_maps


def combine(results):
    diag = np.concatenate([r["out"][:, 0] for r in results])
    rsum = np.concatenate([r["out"][:, 1] for r in results])
    logp_diag = diag - np.log(rsum)
    return np.asarray(-np.mean(logp_diag), dtype=np.float32)


def _ensure_ntff_hook():
    """The agent image's antenv lacks axon_hooks; synthesize it and register the
    ctypes NTFF hook from trn_agent_boot so trace=True profiling works."""
    import sys
    import types

    try:
        from antenv.axon_hooks import get_axon_ntff_profile_hook  # noqa: F401

        return
    except ImportError:
        pass
    import antenv
    from trn_agent_boot.trn_boot import _ntff_profile_via_ctypes

    mod = types.ModuleType("antenv.axon_hooks")
    state = {"hook": None}
    mod.set_axon_ntff_profile_hook = lambda h: state.__setitem__("hook", h)
    mod.get_axon_ntff_profile_hook = lambda: state["hook"]
    sys.modules["antenv.axon_hooks"] = mod
    antenv.axon_hooks = mod
    mod.set_axon_ntff_profile_hook(_ntff_profile_via_ctypes("/opt/axon/libaxon_pjrt.so"))


def run_on_hw(in_maps, trace=False):
    from concourse import bass_utils

    if trace:
        _ensure_ntff_hook()
    return bass_utils.run_bass_kernel_spmd(
        get_nc(), in_maps, core_ids=list(range(M)), trace=trace
    )


def kernel(face_j, ner_j):
    res = run_on_hw(build_in_maps(face_j, ner_j))
    return combine(res.results)


# revision 8
# speedup vs baseline: 1.8414x; 1.8414x over previous
"""Trainium2 Bass kernel for nn_BatchSoftmaxNomax (batch contrastive softmax loss).

Math: scores[b,c,n,f] = <ner[b,n,:], face[c,f,:]>, logits = scores.mean((n,f)),
loss = -mean_b log_softmax(logits)[b,b].
Since the span-means are linear, logits[b,c] = <mean_n ner[b], mean_f face[c]>,
so the O(B^2*N^2*D) einsum collapses to two mean-reductions + a [B,D]x[D,B] matmul.

Sharding (8 cores, batch-sharded), two launches with a host-side gather between
them (a device AllGather works but costs 35-60us of cross-rank barrier/launch-skew
wait through this runtime, dwarfing the 5us of exchanged data):

Launch A (per core, 32 batch rows): stream the ner/face slices (2 MB each),
span-mean them via PE matmuls against a 1/32 selection matrix (fp32r), transpose
the means on the PE to [d, batch] layout, emit fmt/nmt [128, 4*32].
Host: concatenate the 8 cores' face-mean transposes into fmt_full [128, 4, 256].
Launch B (per core): 4 accumulating fp32r matmuls give its [32, 256] logit rows;
ACT exp with fused row-sum accumulate and a DVE masked reduce give per-row
(diag logit, sum exp); host: loss = -mean(diag - log(rowsum)).
"""

import numpy as np
from contextlib import ExitStack

B = 256      # global batch
N1 = 32      # ner spans
N2 = 32      # face spans
D = 512      # embed dim
M = 8        # cores
BL = B // M  # local batch rows per core (32)
R = BL * N1  # rows of the flattened local slice (1024)
PJ = R // 128  # rows per partition in the [128, PJ, D] DMA view (8)
NCH = 4      # DMA chunks per input tensor
JPC = PJ // NCH
KD = D // 128  # d-chunks (4)

_CACHE = {}


def _emit_a(ctx, tc, fmt_out, nmt_out, ner, face, sel, ident):
    from concourse import mybir

    nc = tc.nc
    f32 = mybir.dt.float32
    f32r = mybir.dt.float32r

    consts = ctx.enter_context(tc.tile_pool(name="consts", bufs=1))
    chunks = ctx.enter_context(tc.tile_pool(name="chunks", bufs=4))
    sbuf = ctx.enter_context(tc.tile_pool(name="work", bufs=1))
    mpsum = ctx.enter_context(tc.tile_pool(name="mpsum", bufs=2, space="PSUM"))
    tpsum = ctx.enter_context(tc.tile_pool(name="tpsum", bufs=2, space="PSUM"))

    sel_sb = consts.tile([128, BL], f32r)
    nc.gpsimd.dma_start(sel_sb[:], sel)
    ident_sb = consts.tile([BL, BL], f32)
    nc.sync.dma_start(ident_sb[:], ident)

    def mean_t(src_ap, out_dram, tag):
        # psum[m, d] = sum_{p,j} sel[p, m] * src[8p + j, d] = (1/32) sum_n src[32m + n, d]
        view = src_ap.rearrange("(p j) d -> p j d", p=128)
        ps = mpsum.tile([BL, D], f32, tag="mean", bufs=2)
        for q in range(NCH):
            t = chunks.tile([128, JPC, D], f32r, tag="chunk", bufs=4)
            nc.gpsimd.dma_start(t[:], view[:, q * JPC:(q + 1) * JPC, :])
            for jj in range(JPC):
                j = q * JPC + jj
                nc.tensor.matmul(
                    ps[:], sel_sb[:], t[:, jj, :],
                    start=(j == 0), stop=(j == PJ - 1),
                )
        mn = sbuf.tile([BL, D], f32, tag="mean_sb_" + tag)
        nc.vector.tensor_copy(mn[:], ps[:])
        # transpose to [d, batch]: out[:, 32k + i] = mn[i, 128k + d']
        tp = tpsum.tile([128, KD * BL], f32, tag="tp_" + tag)
        for k in range(KD):
            nc.tensor.transpose(
                tp[:, k * BL:(k + 1) * BL], mn[:, k * 128:(k + 1) * 128], ident_sb[:]
            )
        ts = sbuf.tile([128, KD * BL], f32, tag="tps_" + tag)
        nc.vector.tensor_copy(ts[:], tp[:])
        nc.sync.dma_start(out_dram, ts[:])

    mean_t(face, fmt_out, "fm")
    mean_t(ner, nmt_out, "nm")


def _emit_b(ctx, tc, out, fmt_full, nmt, dmask):
    from concourse import mybir

    nc = tc.nc
    f32 = mybir.dt.float32
    f32r = mybir.dt.float32r
    AF = mybir.ActivationFunctionType
    ALU = mybir.AluOpType

    sbuf = ctx.enter_context(tc.tile_pool(name="work", bufs=1))
    lpsum = ctx.enter_context(tc.tile_pool(name="lpsum", bufs=1, space="PSUM"))

    # Warm the ACT exp table set while DMAs stream.
    warm_in = sbuf.tile([1, 1], f32)
    nc.vector.memset(warm_in[:], 0.0)
    warm_out = sbuf.tile([1, 1], f32)
    nc.scalar.activation(warm_out[:], warm_in[:], AF.Exp)

    ff = sbuf.tile([128, KD, B], f32r)
    nc.gpsimd.dma_start(ff[:], fmt_full)
    nt = sbuf.tile([128, KD * BL], f32r)
    nc.gpsimd.dma_start(nt[:], nmt)
    dm = sbuf.tile([BL, B], f32)
    nc.sync.dma_start(dm[:], dmask)

    lg = lpsum.tile([BL, B], f32)
    for k in range(KD):
        nc.tensor.matmul(
            lg[:], nt[:, k * BL:(k + 1) * BL], ff[:, k, :],
            start=(k == 0), stop=(k == KD - 1),
        )

    # res[:, 1] = sum_c exp(logits); res[:, 0] = logits[b, 32*core + b] via mask.
    # NB: InstTensorTensorReduce and 2-input DVE reads from PSUM wedge the device
    # here, so: ACT exp straight off PSUM (fine), diag via copy -> mult -> reduce.
    res = sbuf.tile([BL, 2], f32)
    e_sb = sbuf.tile([BL, B], f32)
    nc.scalar.activation(e_sb[:], lg[:], AF.Exp, accum_out=res[:, 1:2])
    lg_sb = sbuf.tile([BL, B], f32)
    nc.vector.tensor_copy(lg_sb[:], lg[:])
    dscratch = sbuf.tile([BL, B], f32)
    nc.vector.tensor_tensor(out=dscratch[:], in0=lg_sb[:], in1=dm[:], op=ALU.mult)
    nc.vector.tensor_reduce(
        out=res[:, 0:1], in_=dscratch[:], axis=mybir.AxisListType.X, op=ALU.add
    )
    nc.sync.dma_start(out, res[:])


def _build_a():
    import concourse.tile as tile
    from concourse import bacc, mybir

    f32 = mybir.dt.float32
    nc = bacc.Bacc("TRN2", target_bir_lowering=False, debug=False, num_devices=M)
    ner = nc.dram_tensor("ner", [R, D], f32, kind="ExternalInput").ap()
    face = nc.dram_tensor("face", [R, D], f32, kind="ExternalInput").ap()
    sel = nc.dram_tensor("sel", [128, BL], f32, kind="ExternalInput").ap()
    ident = nc.dram_tensor("ident", [BL, BL], f32, kind="ExternalInput").ap()
    fmt = nc.dram_tensor("fmt", [128, KD * BL], f32, kind="ExternalOutput").ap()
    nmt = nc.dram_tensor("nmt", [128, KD * BL], f32, kind="ExternalOutput").ap()
    with tile.TileContext(nc) as tc:
        with ExitStack() as ctx:
            _emit_a(ctx, tc, fmt, nmt, ner, face, sel, ident)
    nc.compile()
    return nc


def _build_b():
    import concourse.tile as tile
    from concourse import bacc, mybir

    f32 = mybir.dt.float32
    nc = bacc.Bacc("TRN2", target_bir_lowering=False, debug=False, num_devices=M)
    fmt_full = nc.dram_tensor("fmt_full", [128, KD, B], f32, kind="ExternalInput").ap()
    nmt = nc.dram_tensor("nmt", [128, KD * BL], f32, kind="ExternalInput").ap()
    dmask = nc.dram_tensor("dmask", [BL, B], f32, kind="ExternalInput").ap()
    out = nc.dram_tensor("out", [BL, 2], f32, kind="ExternalOutput").ap()
    with tile.TileContext(nc) as tc:
        with ExitStack() as ctx:
            _emit_b(ctx, tc, out, fmt_full, nmt, dmask)
    nc.compile()
    return nc


def get_nc_a():
    if "a" not in _CACHE:
        _CACHE["a"] = _build_a()
    return _CACHE["a"]


def get_nc_b():
    if "b" not in _CACHE:
        _CACHE["b"] = _build_b()
    return _CACHE["b"]


def build_in_maps_a(face_j, ner_j):
    face_j = np.ascontiguousarray(face_j, dtype=np.float32)
    ner_j = np.ascontiguousarray(ner_j, dtype=np.float32)
    sel = np.zeros((128, BL), np.float32)
    sel[np.arange(128), np.arange(128) // 4] = 1.0 / N1
    ident = np.eye(BL, dtype=np.float32)
    return [
        {
            "ner": np.ascontiguousarray(ner_j[c * BL:(c + 1) * BL].reshape(R, D)),
            "face": np.ascontiguousarray(face_j[c * BL:(c + 1) * BL].reshape(R, D)),
            "sel": sel,
            "ident": ident,
        }
        for c in range(M)
    ]


def build_in_maps_b(results_a):
    # fmt[c][d', 32k + i] -> fmt_full[d', k, 32c + i]
    F = np.stack([r["fmt"] for r in results_a])        # [8, 128, 4*32]
    F = F.reshape(M, 128, KD, BL)                      # [c, d', k, i]
    fmt_full = np.ascontiguousarray(
        F.transpose(1, 2, 0, 3).reshape(128, KD, B), dtype=np.float32
    )
    in_maps = []
    for c in range(M):
        dm = np.zeros((BL, B), np.float32)
        dm[np.arange(BL), c * BL + np.arange(BL)] = 1.0
        in_maps.append(
            {
                "fmt_full": fmt_full,
                "nmt": np.ascontiguousarray(results_a[c]["nmt"]),
                "dmask": dm,
            }
        )
    return in_maps


def combine(results_b):
    diag = np.concatenate([r["out"][:, 0] for r in results_b])
    rsum = np.concatenate([r["out"][:, 1] for r in results_b])
    return np.asarray(-np.mean(diag - np.log(rsum)), dtype=np.float32)


def _ensure_ntff_hook():
    """The agent image's antenv lacks axon_hooks; synthesize it and register the
    ctypes NTFF hook from trn_agent_boot so trace=True profiling works."""
    import sys
    import types

    try:
        from antenv.axon_hooks import get_axon_ntff_profile_hook  # noqa: F401

        return
    except ImportError:
        pass
    import antenv
    from trn_agent_boot.trn_boot import _ntff_profile_via_ctypes

    mod = types.ModuleType("antenv.axon_hooks")
    state = {"hook": None}
    mod.set_axon_ntff_profile_hook = lambda h: state.__setitem__("hook", h)
    mod.get_axon_ntff_profile_hook = lambda: state["hook"]
    sys.modules["antenv.axon_hooks"] = mod
    antenv.axon_hooks = mod
    mod.set_axon_ntff_profile_hook(_ntff_profile_via_ctypes("/opt/axon/libaxon_pjrt.so"))


def run_stage(nc, in_maps, trace=False, **kw):
    from concourse import bass_utils

    if trace:
        _ensure_ntff_hook()
    return bass_utils.run_bass_kernel_spmd(
        nc, in_maps, core_ids=list(range(M)), trace=trace, **kw
    )


def kernel(face_j, ner_j):
    res_a = run_stage(get_nc_a(), build_in_maps_a(face_j, ner_j))
    res_b = run_stage(get_nc_b(), build_in_maps_b(res_a.results))
    return combine(res_b.results)
